# revision 12
# baseline (speedup 1.0000x reference)
"""MiniBatchDiscrimination kernel for 8 Trainium2 NeuronCores.

Math: m = (x @ T).reshape(B, K, D); l1[i,k,j] = sum_d |m[i,k,d]-m[j,k,d]|;
feat[i,k] = sum_j exp(-l1[i,k,j]); out = concat([x, feat], axis=1).

Sharding: data-parallel over i-rows (128 per core). Every core computes the
full projection m.T = (x @ T).T on its own PE (redundant but cheap, ~14us,
avoids collectives), plus m restricted to its own 128 rows, then evaluates
its [128, K, B] slice of the pairwise kernel.

Per (k, d) the PE produces diff[i, j] = mT[kd, j] - m_mine[i, kd] straight
into PSUM as one contraction-2 matmul: lhsT = [ones_row; m_mine_col],
rhs = [mT_row; -ones_row]. PE operands must start at partition 0, so the
per-kd rows are staged to partitions 0/1 of small SBUF staging tiles by DMA
(groups of 8 kd at a time, double-buffered; the constant ones/-ones rows are
re-filled from DRAM alongside). Then |diff| -> fp16 on ScalarE (Abs) for 3
of 5 dims and on VectorE (tensor_scalar abs_max vs 0) for 2, VectorE
accumulates l1 over d in fp16, and ScalarE Exp(scale=-1) with accum_out
fuses exp with the j-reduction.
"""

import numpy as np
from contextlib import ExitStack

import concourse.bass as bass
import concourse.tile as tile
from concourse import mybir
from concourse.bass_utils import run_bass_kernel_spmd
from concourse.masks import make_identity

B, F = 1024, 1024
K, D = 100, 5
KD = K * D            # 500
NCORES = 8
RPC = B // NCORES     # 128 i-rows per core
KDB = 128             # kd rows per projection block
NKDB = (KD + KDB - 1) // KDB   # 4 (last block 116 rows)
G = 8                 # kd entries per staging refill group
NG = (KD + G - 1) // G         # 63
FP32 = mybir.dt.float32
FP16 = mybir.dt.float16
AF = mybir.ActivationFunctionType
ALU = mybir.AluOpType
N_ACT_ABS = 5         # dims whose |.| runs on ScalarE; rest on VectorE


class TC(tile.TileContext):
    """TileContext whose tail puts sem waits on NOPs instead of the Drain.

    The walrus in this container lowers Drain/NOP with a no-sync-struct ISA
    encoding that holds at most one wait, so the stock tail drain (which
    carries one wait per outstanding proc) fails codegen. Emit one NOP per
    proc, each carrying a single wait, before the drain.
    """

    def _drain_and_barrier(self, tick_clock, wait_clock):
        from concourse.vector_clock import ScopedClock, VectorClock

        gc = tick_clock.global_clock
        n = len(gc)
        for p in range(n):
            t = gc[p]
            if t <= 0:
                continue
            vec = [0] * n
            vec[p] = t
            nop_inst = self.nc.sync.nop(nofuse=True)
            wait_clock.add_sem_waits(
                nop_inst.ins, ScopedClock({None: VectorClock(vec)})
            )
        self.nc.sync.drain()
        self.nc.all_engine_barrier()
        popped = self.nc._tile_sem_poison_stack.pop()
        assert popped is self._sem_poison
        self.nc.clear_and_free_semaphores(list(self.sems.allocated().values()))
        self.nc.all_engine_barrier()


def _hoist_excess_waits(nc):
    """Move excess sem waits onto same-engine NOPs inserted just before.

    This container's walrus encodes Matmult (LDWEIGHTS struct) and
    NoOp/Drain with room for a single sync wait; Tile may attach several.
    Keep one wait on the instruction and carry the rest on dedicated NOPs,
    which is sync-equivalent (same engine, program order).
    """
    def limit_for(inst):
        return 1
    for f in nc.m.functions:
        for bb in f.blocks:
            snapshot = list(bb.instructions)
            if not any(
                i.sync_info is not None
                and len(i.sync_info.on_wait) > limit_for(i)
                for i in snapshot
            ):
                continue
            new_list = []
            for inst in snapshot:
                lim = limit_for(inst)
                si = inst.sync_info
                if lim is not None and si is not None and \
                        len(si.on_wait) > lim:
                    waits = list(si.on_wait)
                    for w in waits[:-lim]:
                        bi = nc.engines[inst.engine].nop(nofuse=True)
                        found = False
                        for f2 in nc.m.functions:
                            for bb2 in f2.blocks:
                                tail = bb2.instructions
                                if tail and tail[-1].name == bi.ins.name:
                                    tail.pop()
                                    found = True
                                    break
                            if found:
                                break
                        assert found, bi.ins.name
                        bi.ins.sync_info = mybir.SyncInfo(
                            on_wait=[w], on_update=[])
                        new_list.append(bi.ins)
                    inst.sync_info = mybir.SyncInfo(
                        on_wait=waits[-lim:], on_update=list(si.on_update))
                new_list.append(inst)
            bb.instructions = new_list


def build_nc(reps: int = 1):
    nc = bass.Bass()
    x_d = nc.dram_tensor("x", [B, F], FP32, kind="ExternalInput")
    t_d = nc.dram_tensor("t", [F, KD], FP32, kind="ExternalInput")
    xm_d = nc.dram_tensor("xm", [RPC, F], FP32, kind="ExternalInput")
    ones_d = nc.dram_tensor("ones1", [1, G * 128], FP32, kind="ExternalInput")
    neg_d = nc.dram_tensor("negones", [1, G * 1024], FP32,
                           kind="ExternalInput")
    out_d = nc.dram_tensor("out", [RPC, F + K], FP32, kind="ExternalOutput")

    with TC(nc) as tc, ExitStack() as ctx:
        const = ctx.enter_context(tc.tile_pool(name="const", bufs=1))

        feat = const.tile([RPC, K], FP32, tag="feat")
        mT = [const.tile([KDB, B], FP32, tag=f"mT{b}", name=f"mT{b}")
              for b in range(NKDB)]
        mTm = [const.tile([KDB, RPC], FP32, tag=f"mTm{b}", name=f"mTm{b}")
               for b in range(NKDB)]

        with ExitStack() as setup_ctx:
            setup_sb = setup_ctx.enter_context(
                tc.tile_pool(name="setup_sb", bufs=3))
            tpsum = setup_ctx.enter_context(
                tc.tile_pool(name="tpsum", bufs=4, space="PSUM"))
            gpsum = setup_ctx.enter_context(
                tc.tile_pool(name="gpsum", bufs=2, space="PSUM"))
            scon = setup_ctx.enter_context(
                tc.tile_pool(name="scon", bufs=1))

            ident = scon.tile([128, 128], FP32, tag="ident")
            make_identity(nc, ident)

            # T in 8 f-blocks [128, KD] (lhsT for the projection GEMMs)
            tT = []
            for fb in range(8):
                tt = scon.tile([128, KD], FP32, tag=f"tT{fb}")
                nc.sync.dma_start(tt[:], t_d[fb * 128:(fb + 1) * 128, :])
                tT.append(tt)

            # x transposed into 8 f-block tiles xT[fb] = [128(f), B(j)]
            xT = [scon.tile([128, B], FP32, tag=f"xT{fb}", name=f"xT{fb}")
                  for fb in range(8)]
            for jb in range(8):
                xin = setup_sb.tile([128, F], FP32, tag="xin")
                nc.sync.dma_start(xin[:], x_d[jb * 128:(jb + 1) * 128, :])
                for fb in range(8):
                    ps = tpsum.tile([128, 128], FP32, tag="tps")
                    nc.tensor.transpose(
                        ps[:], xin[:, fb * 128:(fb + 1) * 128], ident[:])
                    nc.vector.tensor_copy(
                        xT[fb][:, jb * 128:(jb + 1) * 128], ps[:])

            # this core's rows: load, emit x-part of the output, transpose
            xmT = scon.tile([128, F], FP32, tag="xmT")
            xm_in = setup_sb.tile([RPC, F], FP32, tag="xmin")
            nc.sync.dma_start(xm_in[:], xm_d[:])
            nc.sync.dma_start(out_d[:, 0:F], xm_in[:])
            for fb in range(8):
                ps = tpsum.tile([128, 128], FP32, tag="tps")
                nc.tensor.transpose(
                    ps[:], xm_in[:, fb * 128:(fb + 1) * 128], ident[:])
                nc.vector.tensor_copy(xmT[:, fb * 128:(fb + 1) * 128], ps[:])

            # mT[b] = (x @ T).T block  [kdn, B]
            # mTm[b] = (xm @ T).T block [kdn, RPC]
            for b in range(NKDB):
                kd0 = b * KDB
                kdn = min(KDB, KD - kd0)
                for jh in range(2):
                    ps = gpsum.tile([KDB, 512], FP32, tag="gps")
                    for fb in range(8):
                        nc.tensor.matmul(
                            ps[:kdn, :],
                            tT[fb][:, kd0:kd0 + kdn],
                            xT[fb][:, jh * 512:(jh + 1) * 512],
                            start=(fb == 0), stop=(fb == 7))
                    nc.vector.tensor_copy(
                        mT[b][0:kdn, jh * 512:(jh + 1) * 512], ps[:kdn, :])
                ps2 = gpsum.tile([KDB, RPC], FP32, tag="gps2")
                for fb in range(8):
                    nc.tensor.matmul(
                        ps2[:kdn, :],
                        tT[fb][:, kd0:kd0 + kdn],
                        xmT[:, fb * 128:(fb + 1) * 128],
                        start=(fb == 0), stop=(fb == 7))
                nc.vector.tensor_copy(mTm[b][0:kdn, :], ps2[:kdn, :])

        # ---- main loop over kernels k ----
        stage_l = ctx.enter_context(tc.tile_pool(name="stage_l", bufs=2))
        stage_r = ctx.enter_context(tc.tile_pool(name="stage_r", bufs=2))
        l1_pool = ctx.enter_context(tc.tile_pool(name="l1", bufs=3))
        abs_pool = ctx.enter_context(tc.tile_pool(name="absd", bufs=6))
        e_pool = ctx.enter_context(tc.tile_pool(name="E", bufs=2))
        dpsum = ctx.enter_context(tc.tile_pool(name="dpsum", bufs=3,
                                               space="PSUM"))

        staged = {}

        def stage_group(g):
            # Stage kd rows [G*g, G*g+n) to partitions 0/1:
            #   sl: p0 = ones,    p1 = m_mine cols (mTm rows, flattened)
            #   sr: p0 = mT rows, p1 = -ones
            if g in staged:
                return staged[g]
            kd0 = G * g
            n = min(G, KD - kd0)
            b, r = divmod(kd0, KDB)
            sl = stage_l.tile([2, G * 128], FP32, name="sl")
            sr = stage_r.tile([2, G * 1024], FP32, name="sr")
            nc.sync.dma_start(sl[0:1, :], ones_d[:, :])
            nc.sync.dma_start(sl[1:2, 0:n * 128], mTm[b][r:r + n, :])
            nc.sync.dma_start(sr[0:1, 0:n * 1024], mT[b][r:r + n, :])
            nc.sync.dma_start(sr[1:2, :], neg_d[:, :])
            staged.clear()
            staged[g] = (sl, sr)
            return sl, sr

        for rep in range(reps):
          staged.clear()
          for k in range(K):
            l1k = l1_pool.tile([RPC, B], FP16)
            for d in range(D):
                kd = k * D + d
                g, o = divmod(kd, G)
                sl, sr = stage_group(g)
                lhs_ap = sl[:, o * 128:(o + 1) * 128]
                ps = dpsum.tile([RPC, B], FP32)
                for jh in range(2):
                    rhs_ap = sr[:, o * 1024 + jh * 512:o * 1024 + jh * 512
                                + 512]
                    nc.tensor.matmul(
                        ps[:, jh * 512:(jh + 1) * 512], lhs_ap, rhs_ap,
                        start=True, stop=True)
                dst = l1k[:] if d == 0 else abs_pool.tile([RPC, B], FP16,
                                                          name="absd")
                if d < N_ACT_ABS:
                    nc.scalar.activation(dst, ps[:], AF.Abs)
                else:
                    raise AssertionError("abs_max path disabled")
                if d > 0:
                    nc.vector.tensor_tensor(
                        l1k[:], l1k[:], dst, op=ALU.add)
            ek = e_pool.tile([RPC, B], FP16)
            nc.scalar.activation(ek, l1k[:], AF.Exp, scale=-1.0,
                                 accum_out=feat[:, k:k + 1])

        nc.sync.dma_start(out_d[:, F:F + K], feat[:])

    _hoist_excess_waits(nc)
    return nc


_NC_CACHE = None


def _get_nc():
    global _NC_CACHE
    if _NC_CACHE is None:
        _NC_CACHE = build_nc()
    return _NC_CACHE


def kernel(x: np.ndarray, T: np.ndarray) -> np.ndarray:
    x = np.ascontiguousarray(np.asarray(x, dtype=np.float32))
    T = np.ascontiguousarray(np.asarray(T, dtype=np.float32))
    assert x.shape == (B, F) and T.shape == (F, KD)
    nc = _get_nc()
    ones1 = np.ones((1, G * 128), dtype=np.float32)
    negones = np.full((1, G * 1024), -1.0, dtype=np.float32)
    in_maps = [
        {"x": x, "t": T, "xm": x[c * RPC:(c + 1) * RPC],
         "ones1": ones1, "negones": negones}
        for c in range(NCORES)
    ]
    res = run_bass_kernel_spmd(nc, in_maps, list(range(NCORES)))
    return np.concatenate([res.results[c]["out"] for c in range(NCORES)],
                          axis=0)


# revision 18
# speedup vs baseline: 10.0637x; 10.0637x over previous
"""MiniBatchDiscrimination kernel for 8 Trainium2 NeuronCores.

Math: m = (x @ T).reshape(B, K, D); l1[i,k,j] = sum_d |m[i,k,d]-m[j,k,d]|;
feat[i,k] = sum_j exp(-l1[i,k,j]); out = concat([x, feat], axis=1).

Sharding: data-parallel over i-rows (128 per core). Every core computes the
full projection m.T = (x @ T).T on its own PE (redundant but cheap, ~14us,
avoids collectives), plus m restricted to its own 128 rows, then evaluates
its [128, K, B] slice of the pairwise kernel.

Per (k, d) the PE produces diff[i, j] = mT[kd, j] - m_mine[i, kd] straight
into PSUM as one contraction-2 matmul: lhsT = [ones_row; m_mine_col],
rhs = [mT_row; -ones_row]. PE operands must start at partition 0, so the
per-kd rows are staged to partitions 0/1 of small SBUF staging tiles by DMA
(groups of 8 kd at a time, double-buffered; the constant ones/-ones rows are
re-filled from DRAM alongside). Then |diff| -> fp16 on ScalarE (Abs) for 3
of 5 dims and on VectorE (tensor_scalar abs_max vs 0) for 2, VectorE
accumulates l1 over d in fp16, and ScalarE Exp(scale=-1) with accum_out
fuses exp with the j-reduction.
"""

import numpy as np
from contextlib import ExitStack

import concourse.bass as bass
import concourse.tile as tile
from concourse import mybir
from concourse.bass_utils import run_bass_kernel_spmd
from concourse.masks import make_identity

B, F = 1024, 1024
K, D = 100, 5
KD = K * D            # 500
NCORES = 8
RPC = B // NCORES     # 128 i-rows per core
KDB = 128             # kd rows per projection block
NKDB = (KD + KDB - 1) // KDB   # 4 (last block 116 rows)
G = 8                 # kd entries per staging refill group
NG = (KD + G - 1) // G         # 63
FP32 = mybir.dt.float32
FP16 = mybir.dt.float16
AF = mybir.ActivationFunctionType
ALU = mybir.AluOpType
N_ACT_ABS = 5         # dims whose |.| runs on ScalarE; rest on VectorE


class TC(tile.TileContext):
    """TileContext whose tail puts sem waits on NOPs instead of the Drain.

    The walrus in this container lowers Drain/NOP with a no-sync-struct ISA
    encoding that holds at most one wait, so the stock tail drain (which
    carries one wait per outstanding proc) fails codegen. Emit one NOP per
    proc, each carrying a single wait, before the drain.
    """

    def _drain_and_barrier(self, tick_clock, wait_clock):
        from concourse.vector_clock import ScopedClock, VectorClock

        gc = tick_clock.global_clock
        n = len(gc)
        for p in range(n):
            t = gc[p]
            if t <= 0:
                continue
            vec = [0] * n
            vec[p] = t
            nop_inst = self.nc.sync.nop(nofuse=True)
            wait_clock.add_sem_waits(
                nop_inst.ins, ScopedClock({None: VectorClock(vec)})
            )
        self.nc.sync.drain()
        self.nc.all_engine_barrier()
        popped = self.nc._tile_sem_poison_stack.pop()
        assert popped is self._sem_poison
        self.nc.clear_and_free_semaphores(list(self.sems.allocated().values()))
        self.nc.all_engine_barrier()


def _hoist_excess_waits(nc):
    """Move excess sem waits onto same-engine NOPs inserted just before.

    This container's walrus encodes Matmult (LDWEIGHTS struct) and
    NoOp/Drain with room for a single sync wait; Tile may attach several.
    Keep one wait on the instruction and carry the rest on dedicated NOPs,
    which is sync-equivalent (same engine, program order).
    """
    def limit_for(inst):
        return 1
    for f in nc.m.functions:
        for bb in f.blocks:
            snapshot = list(bb.instructions)
            if not any(
                i.sync_info is not None
                and len(i.sync_info.on_wait) > limit_for(i)
                for i in snapshot
            ):
                continue
            new_list = []
            for inst in snapshot:
                lim = limit_for(inst)
                si = inst.sync_info
                if lim is not None and si is not None and \
                        len(si.on_wait) > lim:
                    waits = list(si.on_wait)
                    for w in waits[:-lim]:
                        bi = nc.engines[inst.engine].nop(nofuse=True)
                        found = False
                        for f2 in nc.m.functions:
                            for bb2 in f2.blocks:
                                tail = bb2.instructions
                                if tail and tail[-1].name == bi.ins.name:
                                    tail.pop()
                                    found = True
                                    break
                            if found:
                                break
                        assert found, bi.ins.name
                        bi.ins.sync_info = mybir.SyncInfo(
                            on_wait=[w], on_update=[])
                        new_list.append(bi.ins)
                    inst.sync_info = mybir.SyncInfo(
                        on_wait=waits[-lim:], on_update=list(si.on_update))
                new_list.append(inst)
            bb.instructions = new_list


def build_nc(reps: int = 1):
    nc = bass.Bass()
    x_d = nc.dram_tensor("x", [B, F], FP32, kind="ExternalInput")
    t_d = nc.dram_tensor("t", [F, KD], FP32, kind="ExternalInput")
    xm_d = nc.dram_tensor("xm", [RPC, F], FP32, kind="ExternalInput")
    out_d = nc.dram_tensor("out", [RPC, F + K], FP32, kind="ExternalOutput")

    with TC(nc) as tc, ExitStack() as ctx:
        const = ctx.enter_context(tc.tile_pool(name="const", bufs=1))

        feat = const.tile([RPC, K], FP32, tag="feat")
        mT = [const.tile([KDB, B], FP32, tag=f"mT{b}", name=f"mT{b}")
              for b in range(NKDB)]
        mTm = [const.tile([KDB, RPC], FP32, tag=f"mTm{b}", name=f"mTm{b}")
               for b in range(NKDB)]

        with ExitStack() as setup_ctx:
            setup_sb = setup_ctx.enter_context(
                tc.tile_pool(name="setup_sb", bufs=3))
            tpsum = setup_ctx.enter_context(
                tc.tile_pool(name="tpsum", bufs=4, space="PSUM"))
            gpsum = setup_ctx.enter_context(
                tc.tile_pool(name="gpsum", bufs=2, space="PSUM"))
            scon = setup_ctx.enter_context(
                tc.tile_pool(name="scon", bufs=1))

            ident = scon.tile([128, 128], FP32, tag="ident")
            make_identity(nc, ident)

            # T in 8 f-blocks [128, KD] (lhsT for the projection GEMMs)
            tT = []
            for fb in range(8):
                tt = scon.tile([128, KD], FP32, tag=f"tT{fb}")
                nc.sync.dma_start(tt[:], t_d[fb * 128:(fb + 1) * 128, :])
                tT.append(tt)

            # x transposed into 8 f-block tiles xT[fb] = [128(f), B(j)]
            xT = [scon.tile([128, B], FP32, tag=f"xT{fb}", name=f"xT{fb}")
                  for fb in range(8)]
            for jb in range(8):
                xin = setup_sb.tile([128, F], FP32, tag="xin")
                nc.sync.dma_start(xin[:], x_d[jb * 128:(jb + 1) * 128, :])
                for fb in range(8):
                    ps = tpsum.tile([128, 128], FP32, tag="tps")
                    nc.tensor.transpose(
                        ps[:], xin[:, fb * 128:(fb + 1) * 128], ident[:])
                    nc.vector.tensor_copy(
                        xT[fb][:, jb * 128:(jb + 1) * 128], ps[:])

            # this core's rows: load, emit x-part of the output, transpose
            xmT = scon.tile([128, F], FP32, tag="xmT")
            xm_in = setup_sb.tile([RPC, F], FP32, tag="xmin")
            nc.sync.dma_start(xm_in[:], xm_d[:])
            nc.sync.dma_start(out_d[:, 0:F], xm_in[:])
            for fb in range(8):
                ps = tpsum.tile([128, 128], FP32, tag="tps")
                nc.tensor.transpose(
                    ps[:], xm_in[:, fb * 128:(fb + 1) * 128], ident[:])
                nc.vector.tensor_copy(xmT[:, fb * 128:(fb + 1) * 128], ps[:])

            # mT[b] = (x @ T).T block  [kdn, B]
            # mTm[b] = (xm @ T).T block [kdn, RPC]
            for b in range(NKDB):
                kd0 = b * KDB
                kdn = min(KDB, KD - kd0)
                for jh in range(2):
                    ps = gpsum.tile([KDB, 512], FP32, tag="gps")
                    for fb in range(8):
                        nc.tensor.matmul(
                            ps[:kdn, :],
                            tT[fb][:, kd0:kd0 + kdn],
                            xT[fb][:, jh * 512:(jh + 1) * 512],
                            start=(fb == 0), stop=(fb == 7))
                    nc.vector.tensor_copy(
                        mT[b][0:kdn, jh * 512:(jh + 1) * 512], ps[:kdn, :])
                ps2 = gpsum.tile([KDB, RPC], FP32, tag="gps2")
                for fb in range(8):
                    nc.tensor.matmul(
                        ps2[:kdn, :],
                        tT[fb][:, kd0:kd0 + kdn],
                        xmT[:, fb * 128:(fb + 1) * 128],
                        start=(fb == 0), stop=(fb == 7))
                nc.vector.tensor_copy(mTm[b][0:kdn, :], ps2[:kdn, :])

        # ---- main loop over kernels k ----
        sl_tiles = [const.tile([2, G * 128], FP32, tag=f"sl{i}",
                               name=f"sl{i}") for i in range(2)]
        sr_tiles = [const.tile([2, G * 1024], FP32, tag=f"sr{i}",
                               name=f"sr{i}") for i in range(2)]
        for i in range(2):
            nc.vector.memset(sl_tiles[i][0:2, :], 1.0)
            nc.vector.memset(sr_tiles[i][0:2, :], -1.0)
        l1_pool = ctx.enter_context(tc.tile_pool(name="l1", bufs=3))
        abs_pool = ctx.enter_context(tc.tile_pool(name="absd", bufs=6))
        e_pool = ctx.enter_context(tc.tile_pool(name="E", bufs=2))
        dpsum = ctx.enter_context(tc.tile_pool(name="dpsum", bufs=3,
                                               space="PSUM"))

        staged = {}

        def stage_group(g):
            # Stage kd rows [G*g, G*g+n) to partitions 0/1:
            #   sl: p0 = ones,    p1 = m_mine cols (mTm rows, flattened)
            #   sr: p0 = mT rows, p1 = -ones
            if g in staged:
                return staged[g]
            kd0 = G * g
            n = min(G, KD - kd0)
            b, r = divmod(kd0, KDB)
            sl = sl_tiles[g % 2]
            sr = sr_tiles[g % 2]
            nc.gpsimd.dma_start(sl[1:2, 0:n * 128], mTm[b][r:r + n, :])
            nc.gpsimd.dma_start(sr[0:1, 0:n * 1024], mT[b][r:r + n, :])
            staged.clear()
            staged[g] = (sl, sr)
            return sl, sr

        for rep in range(reps):
          staged.clear()
          for k in range(K):
            l1k = l1_pool.tile([RPC, B], FP16)
            for d in range(D):
                kd = k * D + d
                g, o = divmod(kd, G)
                sl, sr = stage_group(g)
                lhs_ap = sl[:, o * 128:(o + 1) * 128]
                ps = dpsum.tile([RPC, B], FP32)
                for jh in range(2):
                    rhs_ap = sr[:, o * 1024 + jh * 512:o * 1024 + jh * 512
                                + 512]
                    nc.tensor.matmul(
                        ps[:, jh * 512:(jh + 1) * 512], lhs_ap, rhs_ap,
                        start=True, stop=True)
                dst = l1k[:] if d == 0 else abs_pool.tile([RPC, B], FP16,
                                                          name="absd")
                if d < N_ACT_ABS:
                    nc.scalar.activation(dst, ps[:], AF.Abs)
                else:
                    raise AssertionError("abs_max path disabled")
                if d > 0:
                    nc.vector.tensor_tensor(
                        l1k[:], l1k[:], dst, op=ALU.add)
            ek = e_pool.tile([RPC, B], FP16)
            nc.scalar.activation(ek, l1k[:], AF.Exp, scale=-1.0,
                                 accum_out=feat[:, k:k + 1])

        nc.sync.dma_start(out_d[:, F:F + K], feat[:])

    _hoist_excess_waits(nc)
    return nc


_NC_CACHE = None


def _get_nc():
    global _NC_CACHE
    if _NC_CACHE is None:
        _NC_CACHE = build_nc()
    return _NC_CACHE


def kernel(x: np.ndarray, T: np.ndarray) -> np.ndarray:
    x = np.ascontiguousarray(np.asarray(x, dtype=np.float32))
    T = np.ascontiguousarray(np.asarray(T, dtype=np.float32))
    assert x.shape == (B, F) and T.shape == (F, KD)
    nc = _get_nc()
    in_maps = [
        {"x": x, "t": T, "xm": x[c * RPC:(c + 1) * RPC]}
        for c in range(NCORES)
    ]
    res = run_bass_kernel_spmd(nc, in_maps, list(range(NCORES)))
    return np.concatenate([res.results[c]["out"] for c in range(NCORES)],
                          axis=0)
